# revision 1
# baseline (speedup 1.0000x reference)
"""Multi-head attention (B=8, N=1024, C=768, 12 heads) on 8 Trainium2 cores.

Strategy: data-parallel over batch — one batch element per NeuronCore, no
collectives. Per core everything stays on-chip:

  1. qkv projection in two orientations:
       - Q^T/K^T tiles [d3, tok]  (lhsT = w_qkv slices, rhs = x^T)
       - V tiles      [tok, d]    (lhsT = x^T slices,  rhs = w_qkv V-columns)
     Only the head-0/1 Q and K chains run up front; the remaining ten chains
     are spread one-per-head through the attention stream so the PE fills its
     exp-wait gaps and the HAM clock-gate stays warm.
  2. attention per head, software-pipelined (per kt: ST(kt), exp(kt),
     PV(kt-2) + filler matmuls): scores S^T[k, q] = K_h Q_h^T on the PE,
     exp on ScalarE over [128, 1024] tiles (scale=1/8 folded in; no
     max-subtraction — scores are O(5), exp cannot overflow fp32/fp16), then
     PV with a ones-column at position 0 of the V stationary so PSUM row 0
     is the softmax denominator (rows 1:64 are zero padding — PSUM reads
     must start at partition 0 or 64; rows 64:128 are the head output).
     Normalization multiplies rows 64:128 by a GpSimd-broadcast reciprocal
     of row 0.
  3. proj as y^T[c_out, tok] (lhsT = w_proj slices, rhs = attn_out^T tiles,
     per-partition bias); the host transposes back — pure layout.

Numerics: all matmul operands fp16 (10-bit mantissa; scores error shrinks by
the 1/8 softmax scale), fp32 PSUM accumulation and fp32 softmax: measured
~7e-4 max rel err vs the fp32 reference. Dummy matmuls ride out the
input-DMA prologue to keep the PE activity monitor at full clock, and the
qkv projection chains are spread through the attention stream as PE filler.
"""

import numpy as np

import concourse.bass as bass
import concourse.tile as tile
import concourse.mybir as mybir
from concourse import bacc
from concourse.bass_utils import run_bass_kernel_spmd

dt = mybir.dt
AF = mybir.ActivationFunctionType
ALU = mybir.AluOpType

B = 8
C = 768
N = 1024          # tokens per batch element (32*32)
NH = 12           # heads
HD = 64           # head dim
C3 = 3 * C        # 2304
CT = C // 128     # 6 contraction tiles
TT = N // 128     # 8 token tiles
NQH = 2           # q processed in halves of 512 where PSUM-bank-bound
QHW = N // NQH    # 512
SCALE = HD ** -0.5
N_WARMUP_MM = 55  # dummy matmuls riding out the input-DMA prologue


def _build_nc():
    nc = bacc.Bacc(None, target_bir_lowering=False)

    xt_ext = nc.dram_tensor("xt", [C, N], dt.float16, kind="ExternalInput")
    wq_ext = nc.dram_tensor("w_qkv", [C, C3], dt.float16, kind="ExternalInput")
    bqk_ext = nc.dram_tensor("b_qkt", [128, 2 * C // 128], dt.float32, kind="ExternalInput")
    bv_ext = nc.dram_tensor("b_v", [1, C], dt.float32, kind="ExternalInput")
    wp_ext = nc.dram_tensor("w_proj", [C, C], dt.float16, kind="ExternalInput")
    bp_ext = nc.dram_tensor("b_proj", [128, C // 128], dt.float32, kind="ExternalInput")
    y_ext = nc.dram_tensor("y", [C, N], dt.float32, kind="ExternalOutput")

    with (
        tile.TileContext(nc) as tc,
        tc.tile_pool(name="persist", bufs=1) as pp,
        tc.tile_pool(name="qkrot", bufs=3) as qkrot,
        tc.tile_pool(name="ps_big", bufs=2, space="PSUM") as ps_big,
        tc.tile_pool(name="ps_o", bufs=4, space="PSUM") as ps_o,
    ):
        # ---- constants / biases ----
        ones_f32 = pp.tile([128, NH, 1], dt.float32, tag="ones_f32")
        nc.vector.memset(ones_f32[:], 1.0)
        bqk_sb = pp.tile([128, 2 * C // 128], dt.float32, tag="bqk")
        nc.gpsimd.dma_start(out=bqk_sb[:], in_=bqk_ext[:, :])
        bv_sb = pp.tile([128, C], dt.float32, tag="bv")
        nc.gpsimd.dma_start(out=bv_sb[:], in_=bv_ext[0:1, :].to_broadcast((128, C)))
        bp_sb = pp.tile([128, C // 128], dt.float32, tag="bp")
        nc.gpsimd.dma_start(out=bp_sb[:], in_=bp_ext[:, :])

        dummy_sb = pp.tile([128, QHW], dt.float16, tag="dummy_sb")
        nc.vector.memset(dummy_sb[:].bitcast(dt.float32), 0.0)

        # attn_out^T: 6 persistent tiles of [128, N]
        aT = [pp.tile([128, N], dt.float16, name=f"aT{i}", tag=f"aT{i}") for i in range(CT)]
        # V per token-tile [128, NH, 128] fp16: col 0 = ones (softmax
        # denominator row), cols 1:64 zero, cols 64:128 = V for that head.
        v_sb = [pp.tile([128, NH, 128], dt.float16, name=f"v{i}", tag=f"v{i}") for i in range(TT)]

        # rotating Q^T/K^T tiles, keyed by d3-tile index
        qkT = {}

        def qk_chain_thunks(d3):
            """Build one Q^T/K^T projection chain as a list of thunks (12
            matmuls into two half-bank psum chains, then bias -> qkT[d3]) so
            the matmuls can be sprinkled into the attention PE stream."""
            t = qkrot.tile([128, N], dt.float16, tag="qv" if d3 < 6 else "kv",
                           name=f"qkT{d3}")
            qkT[d3] = t
            pss = [ps_o.tile([128, QHW], dt.float32, tag="pov", name=f"ps_qk{d3}_{qh}")
                   for qh in range(NQH)]
            thunks = []

            def mk_mm(qh, ct):
                def run():
                    nc.tensor.matmul(
                        out=pss[qh][:],
                        lhsT=wq_sb[ct][:, 128 * d3:128 * (d3 + 1)],
                        rhs=xt_sb[ct][:, QHW * qh:QHW * (qh + 1)],
                        start=(ct == 0), stop=(ct == CT - 1),
                    )
                return run

            for qh in range(NQH):
                for ct in range(CT):
                    thunks.append(mk_mm(qh, ct))

            def bias():
                for qh in range(NQH):
                    nc.vector.tensor_scalar(
                        out=t[:, QHW * qh:QHW * (qh + 1)], in0=pss[qh][:],
                        scalar1=bqk_sb[:, d3:d3 + 1], scalar2=None, op0=ALU.add,
                    )
            thunks.append(bias)
            return thunks

        def qk_chain(d3):
            for th in qk_chain_thunks(d3):
                th()

        with (
            tc.tile_pool(name="att_sb", bufs=12) as att_sb,
            tc.tile_pool(name="att_small", bufs=3) as att_small,
        ):
            def att_head(h, fillers=()):
                q_tile = qkT[h // 2]
                k_tile = qkT[6 + h // 2]
                po = 64 * (h % 2)
                fillers = list(fillers)
                # Software-pipelined head: per kt emit ST(kt), exp(kt), then
                # PV(kt-2) plus filler matmuls (no exp dependency), so the PE
                # always has ~an exp's worth of ready work per iteration.
                PIPE = 2
                ess = []
                povs = []
                for qh in range(NQH):
                    pov = ps_o.tile([128, QHW], dt.float32, tag="pov", name=f"pov{h}_{qh}")
                    povs.append(pov)
                fi = 0

                def pv_pair(kt):
                    for qh in range(NQH):
                        nc.tensor.matmul(
                            out=povs[qh][:],
                            lhsT=v_sb[kt][:, h, :],
                            rhs=ess[kt][:, QHW * qh:QHW * (qh + 1)],
                            start=(kt == 0), stop=(kt == TT - 1),
                        )

                for kt in range(TT + PIPE):
                    if kt < TT:
                        pss = ps_big.tile([128, N], dt.float32, tag="big", name=f"pss{h}_{kt}")
                        for qh in range(NQH):
                            nc.tensor.matmul(
                                out=pss[:, QHW * qh:QHW * (qh + 1)],
                                lhsT=k_tile[po:po + HD, 128 * kt:128 * (kt + 1)],
                                rhs=q_tile[po:po + HD, QHW * qh:QHW * (qh + 1)],
                                start=True, stop=True,
                            )
                        es = att_sb.tile([128, N], dt.float16, tag="es", name=f"es{h}_{kt}")
                        nc.scalar.activation(out=es[:], in_=pss[:], func=AF.Exp, scale=SCALE)
                        ess.append(es)
                    if kt >= PIPE:
                        # fillers first: anything a PV may consume (e.g. att0's
                        # V tiles) must be emitted before the PV that reads it
                        want = min(len(fillers), -(-(kt * len(fillers)) // TT))
                        while fi < want:
                            fillers[fi]()
                            fi += 1
                        pv_pair(kt - PIPE)
                while fi < len(fillers):
                    fillers[fi]()
                    fi += 1
                # normalize rows 64:128 by reciprocal of denominator row 0
                for qh in range(NQH):
                    r_sb = att_small.tile([1, QHW], dt.float32, tag="r")
                    nc.vector.reciprocal_approx_fast(out=r_sb[:], in_=povs[qh][0:1, :])
                    rb_sb = att_small.tile([HD, QHW], dt.float32, tag="rb")
                    nc.gpsimd.partition_broadcast(rb_sb[:], r_sb[:])
                    nc.vector.tensor_tensor(
                        out=aT[h // 2][po:po + HD, QHW * qh:QHW * (qh + 1)],
                        in0=povs[qh][64:128, :],
                        in1=rb_sb[:],
                        op=ALU.mult,
                    )

            with tc.tile_pool(name="xw", bufs=1) as xw:
                # HAM warm-up: dummy matmuls with no input dependencies
                pwarm = ps_big.tile([128, QHW], dt.float32, tag="big", name="pwarm")
                for _ in range(N_WARMUP_MM):
                    nc.tensor.matmul(
                        out=pwarm[:], lhsT=dummy_sb[:, 0:128], rhs=dummy_sb[:],
                        start=True, stop=True,
                    )

                # load x^T and w_qkv, interleaved so the first chains can chase
                xt_sb, wq_sb = [None] * CT, [None] * CT
                for i in range(CT):
                    t = xw.tile([128, N], dt.float16, tag=f"xt{i}", name=f"xt{i}")
                    nc.gpsimd.dma_start(out=t[:], in_=xt_ext[128 * i:128 * (i + 1), :])
                    xt_sb[i] = t
                    t = xw.tile([128, C3], dt.float16, tag=f"wq{i}", name=f"wq{i}")
                    nc.gpsimd.dma_start(out=t[:], in_=wq_ext[128 * i:128 * (i + 1), :])
                    wq_sb[i] = t

                qk_chain(0)   # Q heads 0/1
                qk_chain(6)   # K heads 0/1

                # V part of the qkv projection, as thunk lists
                def v_chain_thunks(tt):
                    ps = ps_big.tile([128, N], dt.float32, tag="big", name=f"ps_v{tt}")
                    thunks = []

                    def mk_mm(c0, c1, ct):
                        def run():
                            nc.tensor.matmul(
                                out=ps[:, c0:c1],
                                lhsT=xt_sb[ct][:, 128 * tt:128 * (tt + 1)],
                                rhs=wq_sb[ct][:, 2 * C + c0:2 * C + c1],
                                start=(ct == 0), stop=(ct == CT - 1),
                            )
                        return run

                    for c0, c1 in ((0, 512), (512, C)):
                        for ct in range(CT):
                            thunks.append(mk_mm(c0, c1, ct))

                    def finish():
                        nc.vector.memset(v_sb[tt][:].bitcast(dt.float32), 0.0)
                        nc.vector.tensor_tensor(
                            out=v_sb[tt][:, :, 64:128],
                            in0=ps[:, 0:C].rearrange("p (h d) -> p h d", h=NH),
                            in1=bv_sb[:].rearrange("p (h d) -> p h d", h=NH),
                            op=ALU.add,
                        )
                        nc.vector.tensor_copy(out=v_sb[tt][:, :, 0:1], in_=ones_f32[:])
                    thunks.append(finish)
                    return thunks

                # V tiles 0..1 up front; V 2..7 ride att0's filler slots
                for tt in range(2):
                    for th in v_chain_thunks(tt):
                        th()

                # attention heads 0..9 with remaining qkv work sprinkled into
                # each head's PE stream: att0 carries V tiles 4..7; att(h) for
                # h=1..9 carries one Q/K chain (pair j delivered during heads
                # 2j-1 and 2j, consumed from head 2j+2 on)
                filler_plan = {
                    0: lambda: [t for tt in range(2, TT) for t in v_chain_thunks(tt)],
                    1: lambda: qk_chain_thunks(1) + qk_chain_thunks(7),
                    2: lambda: qk_chain_thunks(2),
                    3: lambda: qk_chain_thunks(8),
                    4: lambda: qk_chain_thunks(3),
                    5: lambda: qk_chain_thunks(9),
                    6: lambda: qk_chain_thunks(4),
                    7: lambda: qk_chain_thunks(10),
                    8: lambda: qk_chain_thunks(5),
                    9: lambda: qk_chain_thunks(11),
                }
                for h in range(10):
                    att_head(h, filler_plan[h]() if h in filler_plan else ())

            # xw freed; last two heads + projection
            with (
                tc.tile_pool(name="wp_pool", bufs=1) as wpp,
                tc.tile_pool(name="y_sb", bufs=3) as y_pool,
            ):
                wp_sb = []
                for i in range(CT):
                    t = wpp.tile([128, C], dt.float16, name=f"wp{i}", tag=f"wp{i}")
                    nc.gpsimd.dma_start(out=t[:], in_=wp_ext[128 * i:128 * (i + 1), :])
                    wp_sb.append(t)

                for h in (10, 11):
                    att_head(h)

                # ---- output projection (y^T: [c_out, tok]; host untransposes) ----
                for co in range(CT):
                    ps = ps_big.tile([128, N], dt.float32, tag="big", name=f"ps_y{co}")
                    for qh in range(NQH):
                        for ct in range(CT):
                            nc.tensor.matmul(
                                out=ps[:, QHW * qh:QHW * (qh + 1)],
                                lhsT=wp_sb[ct][:, 128 * co:128 * (co + 1)],
                                rhs=aT[ct][:, QHW * qh:QHW * (qh + 1)],
                                start=(ct == 0), stop=(ct == CT - 1),
                            )
                    y_sb = y_pool.tile([128, N], dt.float32, tag="y")
                    nc.vector.tensor_scalar(
                        out=y_sb[:], in0=ps[:],
                        scalar1=bp_sb[:, co:co + 1], scalar2=None, op0=ALU.add,
                    )
                    nc.gpsimd.dma_start(out=y_ext[128 * co:128 * (co + 1), :], in_=y_sb[:])

    nc.compile()
    return nc


_NC_CACHE = {}


def kernel(x, w_qkv, b_qkv, w_proj, b_proj, _trace=False):
    x = np.asarray(x, dtype=np.float32)
    w_qkv = np.asarray(w_qkv, dtype=np.float32)
    b_qkv = np.asarray(b_qkv, dtype=np.float32)
    w_proj = np.asarray(w_proj, dtype=np.float32)
    b_proj = np.asarray(b_proj, dtype=np.float32)

    if "nc" not in _NC_CACHE:
        _NC_CACHE["nc"] = _build_nc()
    nc = _NC_CACHE["nc"]

    # host-side prep (pure layout, no arithmetic)
    # b_qkt: Q/K bias columns laid out per d3-tile: [128, 12]
    b_qkt = np.ascontiguousarray(b_qkv[:2 * C].reshape(2 * C // 128, 128).T)
    w_qkv_h = w_qkv.astype(np.float16)
    w_proj_h = w_proj.astype(np.float16)
    b_v = np.ascontiguousarray(b_qkv[2 * C:].reshape(1, C))
    b_p = np.ascontiguousarray(b_proj.reshape(C // 128, 128).T)

    core_ids = list(range(B))
    in_maps = []
    for b in range(B):
        xt = np.ascontiguousarray(x[b].reshape(N, C).T.astype(np.float16))
        in_maps.append({
            "xt": xt,
            "w_qkv": w_qkv_h,
            "b_qkt": b_qkt,
            "b_v": b_v,
            "w_proj": w_proj_h,
            "b_proj": b_p,
        })

    res = run_bass_kernel_spmd(nc, in_maps, core_ids, trace=_trace)
    if _trace:
        _NC_CACHE["last_result"] = res

    out = np.empty((B, 32, 32, C), dtype=np.float32)
    for b in range(B):
        out[b] = res.results[b]["y"].T.reshape(32, 32, C)
    return out

